# revision 8
# baseline (speedup 1.0000x reference)
"""Trainium2 Bass kernel for nn_MixtureOfSoftMaxACF (mixture-of-softmax attention).

Reference computation per batch element b (B=8, one per NeuronCore):
    pi      = softmax(weight @ mean(qt[b], axis=-1))                      # [m=2]
    A_j     = q_j^T k_j / sqrt(d_k)       (q_j, k_j = d=256-row slices)   # [N, N2]
    attn    = sum_j pi_j * softmax(A_j, axis=-1)                          # [N, N2]
    out     = attn @ vt[b]^T                                              # [N, d_v]

Sharding: data-parallel over B across the 8 cores. Inside each core:
  - QK^T runs in bf16 with native [d_k, N] layouts (lhsT=q, rhs=k).
  - exp on ScalarE with accum_out giving row sums for free (no max
    subtraction needed: |logits| <= ~6).
  - mixture weights pi computed on-device from the q-cast accumulation.
  - mixing on VectorE with per-partition scalars r_j = pi_j / S_j.
  - attn is transposed (DMA xbar or PE) to feed the attn @ v matmul.
"""

import sys

if "/opt/trn_rl_repo" not in sys.path:
    sys.path.insert(0, "/opt/trn_rl_repo")

import numpy as np
from contextlib import ExitStack

import concourse.bacc as bacc
import concourse.bass as bass
import concourse.tile as tile
from concourse import mybir
from concourse.bass_utils import run_bass_kernel_spmd
from concourse.masks import make_identity

B, DK, N = 8, 512, 2048
M = 2                       # mixture components
D = DK // M                 # 256 per-mixture head dim
TEMP = float(DK) ** 0.5     # sqrt(d_k), matching the reference
NCH = N // 128              # 16 n-chunks (and n2-chunks)
DCH = DK // 128             # 4 d_k partition chunks

f32 = mybir.dt.float32
bf16 = mybir.dt.bfloat16
Exp = mybir.ActivationFunctionType.Exp
Copy = mybir.ActivationFunctionType.Copy
MULT = mybir.AluOpType.mult
ADD = mybir.AluOpType.add

TRANSPOSE_MODE = "pe"     # "xbar" (DMA xbar transpose) or "pe" (TensorE transpose)

_NC_CACHE = {}


def _body(ctx, tc, qt, kt, vtT, wT, out_d, attn_d, mode):
    nc = tc.nc

    const = ctx.enter_context(tc.tile_pool(name="const", bufs=1))
    resid = ctx.enter_context(tc.tile_pool(name="resid", bufs=1))
    stage = ctx.enter_context(tc.tile_pool(name="stage", bufs=3))
    epool = ctx.enter_context(tc.tile_pool(name="epool", bufs=4))
    apool = ctx.enter_context(tc.tile_pool(name="apool", bufs=3))
    spool = ctx.enter_context(tc.tile_pool(name="spool", bufs=6))
    opool = ctx.enter_context(tc.tile_pool(name="opool", bufs=3))
    # PSUM: psA 2 halves x [128,1024] (2 banks each) = 4 banks,
    # psT [128,512] = 1-2 banks, psU [128,512] x2 = 2 banks.
    psA_p = ctx.enter_context(tc.tile_pool(name="psA", bufs=2, space="PSUM"))
    psT_p = ctx.enter_context(
        tc.tile_pool(name="psT", bufs=(2 if mode == "pe" else 1), space="PSUM")
    )
    psU_p = ctx.enter_context(tc.tile_pool(name="psU", bufs=2, space="PSUM"))

    # ---- load + cast inputs ----
    wT_sb = const.tile([128, DCH, M], f32)
    nc.sync.dma_start(wT_sb, wT[:, :].rearrange("(c p) m -> p c m", p=128))

    # q: fp32 -> bf16 cast on ScalarE, accumulating per-partition row sums
    # (sum over N) which feed the mixture-weight logits.
    q_bf = resid.tile([128, DCH, N], bf16)
    qs = spool.tile([128, DCH], f32, tag="qs")
    for c in range(DCH):
        st = stage.tile([128, N], f32, tag="stage")
        nc.sync.dma_start(st, qt[bass.ts(c, 128), :])
        nc.scalar.activation(q_bf[:, c, :], st, Copy, accum_out=qs[:, c : c + 1])

    k_bf = resid.tile([128, DCH, N], bf16)
    for c in range(DCH):
        st = stage.tile([128, N], f32, tag="stage")
        nc.sync.dma_start(st, kt[bass.ts(c, 128), :])
        nc.vector.tensor_copy(k_bf[:, c, :], st)

    # vT (host-pretransposed to [N2, d_v]): 16 tiles [128 n2, 512 dv]
    vT_bf = resid.tile([128, NCH, DK], bf16)
    vr = vtT[:, :].rearrange("(g c p) d -> g p c d", g=4, p=128)
    for g in range(4):
        st = stage.tile([128, 4, DK], f32, tag="stage")
        nc.sync.dma_start(st, vr[g])
        nc.vector.tensor_copy(vT_bf[:, g * 4 : (g + 1) * 4, :], st)

    # ---- mixture weights pi ----
    # logits[m] = sum_d qsum[d] * wT[d, m]  (qsum = N * bar_q)
    lg_ps = psT_p.tile([128, 512], f32, tag="psT")
    for c in range(DCH):
        nc.tensor.matmul(
            lg_ps[:1, :M],
            lhsT=qs[:, c : c + 1],
            rhs=wT_sb[:, c, :],
            start=(c == 0),
            stop=(c == DCH - 1),
        )
    e_pi = spool.tile([1, M], f32, tag="epi")
    s_pi = spool.tile([1, 1], f32, tag="spi")
    # pi = softmax(logits / N); |logits/N| is tiny so no max subtraction
    nc.scalar.activation(e_pi, lg_ps[:1, :M], Exp, scale=1.0 / float(N), accum_out=s_pi)
    rs_pi = spool.tile([1, 1], f32, tag="rspi")
    nc.vector.reciprocal(rs_pi, s_pi)
    pi_row = spool.tile([1, M], f32, tag="pirow")
    nc.vector.tensor_scalar_mul(pi_row, e_pi, rs_pi)
    # broadcast pi to all 128 partitions via ones-matmul
    ones_r = const.tile([1, 128], f32)
    nc.vector.memset(ones_r, 1.0)
    pi_ps = psT_p.tile([128, 512], f32, tag="psT")
    nc.tensor.matmul(pi_ps[:, :M], lhsT=ones_r, rhs=pi_row, start=True, stop=True)
    pi_bc = const.tile([128, M], f32)
    nc.vector.tensor_copy(pi_bc, pi_ps[:, :M])

    if mode == "pe":
        ident = const.tile([128, 128], bf16)
        make_identity(nc, ident)

    # ---- main loop over 128-row n-chunks ----
    for ni in range(NCH):
        E = []
        P2 = []
        for h in range(2):
            p2h = spool.tile([128, M], f32, tag=f"P2_{h}", name=f"P2_{h}_{ni}")
            P2.append(p2h)
        for j in range(M):
            e_t = epool.tile([128, N], bf16, tag="E")
            pah = []
            for h in range(2):
                pa = psA_p.tile([128, 1024], f32, tag="psA", name=f"pa_{ni}_{j}_{h}")
                pah.append(pa)
            # d-outer loop: one LDWEIGHTS per (j, d) covers all 4 n2 subtiles
            for d in range(2):
                dch = j * 2 + d
                lhsT = q_bf[:, dch, bass.ts(ni, 128)]
                for h in range(2):
                    for s in range(2):
                        n2o = h * 1024 + s * 512
                        nc.tensor.matmul(
                            pah[h][:, bass.ts(s, 512)],
                            lhsT=lhsT,
                            rhs=k_bf[:, dch, n2o : n2o + 512],
                            start=(d == 0),
                            stop=(d == 1),
                        )
            for h in range(2):
                # exp(logits / temp), bf16 out, with per-partition sum
                nc.scalar.activation(
                    e_t[:, bass.ts(h, 1024)],
                    pah[h],
                    Exp,
                    scale=1.0 / TEMP,
                    accum_out=P2[h][:, j : j + 1],
                )
            E.append(e_t)
        # r_j = pi_j / S_j ; S = P2[0] + P2[1]
        S2 = spool.tile([128, M], f32, tag="S2")
        nc.vector.tensor_add(S2, P2[0], P2[1])
        rS = spool.tile([128, M], f32, tag="rS")
        nc.vector.reciprocal(rS, S2)
        r2 = spool.tile([128, M], f32, tag="r2")
        nc.vector.tensor_mul(r2, rS, pi_bc)
        # attn = r_0 * E_0 + r_1 * E_1
        t_bf = apool.tile([128, N], bf16, tag="tbf")
        nc.vector.tensor_scalar_mul(t_bf, E[1], r2[:, 1:2])
        a_bf = apool.tile([128, N], bf16, tag="abf")
        nc.vector.scalar_tensor_tensor(a_bf, E[0], r2[:, 0:1], t_bf, MULT, ADD)
        a_f32 = apool.tile([128, N], f32, tag="af32")
        # upcast on the otherwise-idle GpSimd engine
        nc.gpsimd.tensor_copy(a_f32, a_bf)
        nc.sync.dma_start(attn_d[bass.ts(ni, 128), :], a_f32)

        # transpose attn row-block for the AV matmul
        aT = apool.tile([128, N], bf16, tag="aT")
        if mode == "pe":
            for g in range(4):
                pt = psT_p.tile([128, 512], bf16, tag="psT")
                for kk in range(4):
                    kb = g * 4 + kk
                    nc.tensor.transpose(
                        pt[:, bass.ts(kk, 128)], a_bf[:, bass.ts(kb, 128)], ident
                    )
                nc.vector.tensor_copy(aT[:, bass.ts(g, 512)], pt)
        else:
            for kb in range(NCH):
                nc.scalar.dma_start(
                    aT[:, bass.ts(kb, 128)], a_bf[:, bass.ts(kb, 128)], transpose=True
                )

        # out[n-chunk] = attn @ v = sum_kb aT_kb^T @ vT_kb
        pu = psU_p.tile([128, DK], f32, tag="psU")
        for kb in range(NCH):
            nc.tensor.matmul(
                pu,
                lhsT=aT[:, bass.ts(kb, 128)],
                rhs=vT_bf[:, kb, :],
                start=(kb == 0),
                stop=(kb == NCH - 1),
            )
        o_t = opool.tile([128, DK], f32, tag="ot")
        nc.scalar.copy(o_t, pu)
        nc.sync.dma_start(out_d[bass.ts(ni, 128), :], o_t)


def _build(mode):
    nc = bacc.Bacc()
    qt = nc.dram_tensor("qt", [DK, N], f32, kind="ExternalInput")
    kt = nc.dram_tensor("kt", [DK, N], f32, kind="ExternalInput")
    vtT = nc.dram_tensor("vtT", [N, DK], f32, kind="ExternalInput")
    wT = nc.dram_tensor("wT", [DK, M], f32, kind="ExternalInput")
    out_d = nc.dram_tensor("out", [N, DK], f32, kind="ExternalOutput")
    attn_d = nc.dram_tensor("attn", [N, N], f32, kind="ExternalOutput")
    with tile.TileContext(nc) as tc, ExitStack() as ctx:
        _body(ctx, tc, qt, kt, vtT, wT, out_d, attn_d, mode)
    nc.finalize()
    return nc


def _get_nc(mode=None):
    mode = mode or TRANSPOSE_MODE
    if mode not in _NC_CACHE:
        _NC_CACHE[mode] = _build(mode)
    return _NC_CACHE[mode]


def _in_maps(qt, kt, vt, weight):
    qt = np.asarray(qt, dtype=np.float32)
    kt = np.asarray(kt, dtype=np.float32)
    vt = np.asarray(vt, dtype=np.float32)
    wT = np.ascontiguousarray(np.asarray(weight, dtype=np.float32).T)  # [DK, M]
    maps = []
    for b in range(B):
        maps.append(
            {
                "qt": np.ascontiguousarray(qt[b]),
                "kt": np.ascontiguousarray(kt[b]),
                "vtT": np.ascontiguousarray(vt[b].T),  # [N2, d_v]
                "wT": wT,
            }
        )
    return maps


def kernel_with_result(qt, kt, vt, weight, **run_kwargs):
    """Like kernel() but also returns the BassKernelResults (for profiling)."""
    nc = _get_nc()
    res = run_bass_kernel_spmd(nc, _in_maps(qt, kt, vt, weight),
                               core_ids=list(range(B)), **run_kwargs)
    output = np.stack([np.asarray(res.results[b]["out"]) for b in range(B)])
    attn = np.stack([np.asarray(res.results[b]["attn"]) for b in range(B)])
    return (output.astype(np.float32, copy=False),
            attn.astype(np.float32, copy=False), res)


def kernel(qt, kt, vt, weight):
    """Full-input entry point: shards over B across 8 cores, returns
    (output [B, N, d_v], attn [B, N, N2]) matching the reference."""
    out, attn, _ = kernel_with_result(qt, kt, vt, weight)
    return out, attn


# revision 9
# speedup vs baseline: 1.2980x; 1.2980x over previous
"""Trainium2 Bass kernel for nn_MixtureOfSoftMaxACF (mixture-of-softmax attention).

Reference computation per batch element b (B=8, one per NeuronCore):
    pi      = softmax(weight @ mean(qt[b], axis=-1))                      # [m=2]
    A_j     = q_j^T k_j / sqrt(d_k)       (q_j, k_j = d=256-row slices)   # [N, N2]
    attn    = sum_j pi_j * softmax(A_j, axis=-1)                          # [N, N2]
    out     = attn @ vt[b]^T                                              # [N, d_v]

Sharding: data-parallel over B across the 8 cores. Inside each core:
  - QK^T runs in bf16 with native [d_k, N] layouts (lhsT=q, rhs=k).
  - exp on ScalarE with accum_out giving row sums for free (no max
    subtraction needed: |logits| <= ~6).
  - mixture weights pi computed on-device from the q-cast accumulation.
  - mixing on VectorE with per-partition scalars r_j = pi_j / S_j.
  - attn is transposed (DMA xbar or PE) to feed the attn @ v matmul.
"""

import sys

if "/opt/trn_rl_repo" not in sys.path:
    sys.path.insert(0, "/opt/trn_rl_repo")

import numpy as np
from contextlib import ExitStack

import concourse.bacc as bacc
import concourse.bass as bass
import concourse.tile as tile
from concourse import mybir
from concourse.bass_utils import run_bass_kernel_spmd
from concourse.masks import make_identity

B, DK, N = 8, 512, 2048
M = 2                       # mixture components
D = DK // M                 # 256 per-mixture head dim
TEMP = float(DK) ** 0.5     # sqrt(d_k), matching the reference
NCH = N // 128              # 16 n-chunks (and n2-chunks)
DCH = DK // 128             # 4 d_k partition chunks

f32 = mybir.dt.float32
bf16 = mybir.dt.bfloat16
Exp = mybir.ActivationFunctionType.Exp
Copy = mybir.ActivationFunctionType.Copy
MULT = mybir.AluOpType.mult
ADD = mybir.AluOpType.add

TRANSPOSE_MODE = "pe"     # "xbar" (DMA xbar transpose) or "pe" (TensorE transpose)

_NC_CACHE = {}


def _body(ctx, tc, qt, kt, vtT, wT, out_d, attn_d, mode):
    nc = tc.nc

    const = ctx.enter_context(tc.tile_pool(name="const", bufs=1))
    resid = ctx.enter_context(tc.tile_pool(name="resid", bufs=1))
    stage = ctx.enter_context(tc.tile_pool(name="stage", bufs=3))
    epool = ctx.enter_context(tc.tile_pool(name="epool", bufs=4))
    apool = ctx.enter_context(tc.tile_pool(name="apool", bufs=3))
    spool = ctx.enter_context(tc.tile_pool(name="spool", bufs=6))
    opool = ctx.enter_context(tc.tile_pool(name="opool", bufs=3))
    # PSUM: psA 2 halves x [128,1024] (2 banks each) = 4 banks,
    # psT [128,512] = 1-2 banks, psU [128,512] x2 = 2 banks.
    psA_p = ctx.enter_context(tc.tile_pool(name="psA", bufs=2, space="PSUM"))
    psT_p = ctx.enter_context(
        tc.tile_pool(name="psT", bufs=(2 if mode == "pe" else 1), space="PSUM")
    )
    psU_p = ctx.enter_context(tc.tile_pool(name="psU", bufs=2, space="PSUM"))

    # ---- load + cast inputs ----
    wT_sb = const.tile([128, DCH, M], f32)
    nc.sync.dma_start(wT_sb, wT[:, :].rearrange("(c p) m -> p c m", p=128))

    # q: fp32 -> bf16 cast on ScalarE, accumulating per-partition row sums
    # (sum over N) which feed the mixture-weight logits.
    q_bf = resid.tile([128, DCH, N], bf16)
    qs = spool.tile([128, DCH], f32, tag="qs")
    for c in range(DCH):
        st = stage.tile([128, N], f32, tag="stage")
        nc.sync.dma_start(st, qt[bass.ts(c, 128), :])
        nc.scalar.activation(q_bf[:, c, :], st, Copy, accum_out=qs[:, c : c + 1])

    k_bf = resid.tile([128, DCH, N], bf16)
    for c in range(DCH):
        st = stage.tile([128, N], f32, tag="stage")
        nc.sync.dma_start(st, kt[bass.ts(c, 128), :])
        nc.vector.tensor_copy(k_bf[:, c, :], st)

    # vT (host-pretransposed to [N2, d_v]): 16 tiles [128 n2, 512 dv]
    vT_bf = resid.tile([128, NCH, DK], bf16)
    vr = vtT[:, :].rearrange("(g c p) d -> g p c d", g=4, p=128)
    for g in range(4):
        st = stage.tile([128, 4, DK], f32, tag="stage")
        nc.sync.dma_start(st, vr[g])
        nc.vector.tensor_copy(vT_bf[:, g * 4 : (g + 1) * 4, :], st)

    # ---- mixture weights pi ----
    # logits[m] = sum_d qsum[d] * wT[d, m]  (qsum = N * bar_q)
    lg_ps = psT_p.tile([128, 512], f32, tag="psT")
    for c in range(DCH):
        nc.tensor.matmul(
            lg_ps[:1, :M],
            lhsT=qs[:, c : c + 1],
            rhs=wT_sb[:, c, :],
            start=(c == 0),
            stop=(c == DCH - 1),
        )
    e_pi = spool.tile([1, M], f32, tag="epi")
    s_pi = spool.tile([1, 1], f32, tag="spi")
    # pi = softmax(logits / N); |logits/N| is tiny so no max subtraction
    nc.scalar.activation(e_pi, lg_ps[:1, :M], Exp, scale=1.0 / float(N), accum_out=s_pi)
    rs_pi = spool.tile([1, 1], f32, tag="rspi")
    nc.vector.reciprocal(rs_pi, s_pi)
    pi_row = spool.tile([1, M], f32, tag="pirow")
    nc.vector.tensor_scalar_mul(pi_row, e_pi, rs_pi)
    # broadcast pi to all 128 partitions via ones-matmul
    ones_r = const.tile([1, 128], f32)
    nc.vector.memset(ones_r, 1.0)
    pi_ps = psT_p.tile([128, 512], f32, tag="psT")
    nc.tensor.matmul(pi_ps[:, :M], lhsT=ones_r, rhs=pi_row, start=True, stop=True)
    pi_bc = const.tile([128, M], f32)
    nc.vector.tensor_copy(pi_bc, pi_ps[:, :M])

    if mode == "pe":
        ident = const.tile([128, 128], bf16)
        make_identity(nc, ident)

    # ---- main loop over 128-row n-chunks ----
    for ni in range(NCH):
        E = []
        P2 = []
        for h in range(2):
            p2h = spool.tile([128, M], f32, tag=f"P2_{h}", name=f"P2_{h}_{ni}")
            P2.append(p2h)
        for j in range(M):
            e_t = epool.tile([128, N], bf16, tag="E")
            pah = []
            for h in range(2):
                pa = psA_p.tile([128, 1024], f32, tag="psA", name=f"pa_{ni}_{j}_{h}")
                pah.append(pa)
            # d-outer loop: one LDWEIGHTS per (j, d) covers all 4 n2 subtiles
            for d in range(2):
                dch = j * 2 + d
                lhsT = q_bf[:, dch, bass.ts(ni, 128)]
                for h in range(2):
                    for s in range(2):
                        n2o = h * 1024 + s * 512
                        nc.tensor.matmul(
                            pah[h][:, bass.ts(s, 512)],
                            lhsT=lhsT,
                            rhs=k_bf[:, dch, n2o : n2o + 512],
                            start=(d == 0),
                            stop=(d == 1),
                        )
            for h in range(2):
                # exp(logits / temp), bf16 out, with per-partition sum
                nc.scalar.activation(
                    e_t[:, bass.ts(h, 1024)],
                    pah[h],
                    Exp,
                    scale=1.0 / TEMP,
                    accum_out=P2[h][:, j : j + 1],
                )
            E.append(e_t)
        # r_j = pi_j / S_j ; S = P2[0] + P2[1]
        S2 = spool.tile([128, M], f32, tag="S2")
        nc.vector.tensor_add(S2, P2[0], P2[1])
        rS = spool.tile([128, M], f32, tag="rS")
        nc.vector.reciprocal(rS, S2)
        r2 = spool.tile([128, M], f32, tag="r2")
        nc.vector.tensor_mul(r2, rS, pi_bc)
        # attn = r_0 * E_0 + r_1 * E_1
        t_bf = apool.tile([128, N], bf16, tag="tbf")
        nc.vector.tensor_scalar_mul(t_bf, E[1], r2[:, 1:2])
        a_bf = apool.tile([128, N], bf16, tag="abf")
        nc.vector.scalar_tensor_tensor(a_bf, E[0], r2[:, 0:1], t_bf, MULT, ADD)
        a_f32 = apool.tile([128, N], f32, tag="af32")
        nc.vector.tensor_copy(a_f32, a_bf)
        nc.sync.dma_start(attn_d[bass.ts(ni, 128), :], a_f32)

        # transpose attn row-block for the AV matmul
        aT = apool.tile([128, N], bf16, tag="aT")
        if mode == "pe":
            for g in range(4):
                pt = psT_p.tile([128, 512], bf16, tag="psT")
                for kk in range(4):
                    kb = g * 4 + kk
                    nc.tensor.transpose(
                        pt[:, bass.ts(kk, 128)], a_bf[:, bass.ts(kb, 128)], ident
                    )
                nc.vector.tensor_copy(aT[:, bass.ts(g, 512)], pt)
        else:
            for kb in range(NCH):
                nc.scalar.dma_start(
                    aT[:, bass.ts(kb, 128)], a_bf[:, bass.ts(kb, 128)], transpose=True
                )

        # out[n-chunk] = attn @ v = sum_kb aT_kb^T @ vT_kb
        pu = psU_p.tile([128, DK], f32, tag="psU")
        for kb in range(NCH):
            nc.tensor.matmul(
                pu,
                lhsT=aT[:, bass.ts(kb, 128)],
                rhs=vT_bf[:, kb, :],
                start=(kb == 0),
                stop=(kb == NCH - 1),
            )
        o_t = opool.tile([128, DK], f32, tag="ot")
        nc.scalar.copy(o_t, pu)
        nc.sync.dma_start(out_d[bass.ts(ni, 128), :], o_t)


def _build(mode):
    nc = bacc.Bacc()
    qt = nc.dram_tensor("qt", [DK, N], f32, kind="ExternalInput")
    kt = nc.dram_tensor("kt", [DK, N], f32, kind="ExternalInput")
    vtT = nc.dram_tensor("vtT", [N, DK], f32, kind="ExternalInput")
    wT = nc.dram_tensor("wT", [DK, M], f32, kind="ExternalInput")
    out_d = nc.dram_tensor("out", [N, DK], f32, kind="ExternalOutput")
    attn_d = nc.dram_tensor("attn", [N, N], f32, kind="ExternalOutput")
    with tile.TileContext(nc) as tc, ExitStack() as ctx:
        _body(ctx, tc, qt, kt, vtT, wT, out_d, attn_d, mode)
    nc.finalize()
    return nc


def _get_nc(mode=None):
    mode = mode or TRANSPOSE_MODE
    if mode not in _NC_CACHE:
        _NC_CACHE[mode] = _build(mode)
    return _NC_CACHE[mode]


def _in_maps(qt, kt, vt, weight):
    qt = np.asarray(qt, dtype=np.float32)
    kt = np.asarray(kt, dtype=np.float32)
    vt = np.asarray(vt, dtype=np.float32)
    wT = np.ascontiguousarray(np.asarray(weight, dtype=np.float32).T)  # [DK, M]
    maps = []
    for b in range(B):
        maps.append(
            {
                "qt": np.ascontiguousarray(qt[b]),
                "kt": np.ascontiguousarray(kt[b]),
                "vtT": np.ascontiguousarray(vt[b].T),  # [N2, d_v]
                "wT": wT,
            }
        )
    return maps


def kernel_with_result(qt, kt, vt, weight, **run_kwargs):
    """Like kernel() but also returns the BassKernelResults (for profiling)."""
    nc = _get_nc()
    res = run_bass_kernel_spmd(nc, _in_maps(qt, kt, vt, weight),
                               core_ids=list(range(B)), **run_kwargs)
    output = np.stack([np.asarray(res.results[b]["out"]) for b in range(B)])
    attn = np.stack([np.asarray(res.results[b]["attn"]) for b in range(B)])
    return (output.astype(np.float32, copy=False),
            attn.astype(np.float32, copy=False), res)


def kernel(qt, kt, vt, weight):
    """Full-input entry point: shards over B across 8 cores, returns
    (output [B, N, d_v], attn [B, N, N2]) matching the reference."""
    out, attn, _ = kernel_with_result(qt, kt, vt, weight)
    return out, attn
